# revision 4
# baseline (speedup 1.0000x reference)
"""EdgeEncoder kernel for Trainium2 (8 NeuronCores, row-sharded).

Reference (per pair (i, j) of an N x N grid):
    out[h, i, j] = (1/n_ij) * sum_l mask[i,j,l] * sum_d feats[idx[i,j,l], d] * W[l, h, d]
with n_ij = max(#valid l, 1), idx in [-1, E-1], -1 = padding.

Device strategy (per core, which owns 64 rows i):
  - A projected table TBL[c, :] = [T_0[c-1,0..7], ..., T_4[c-1,0..7], valid,
    pad] (64 f32 = 256B rows) is built on PE (79 chunked matmuls of
    featsT17^T @ WV) and written to an HBM scratch tensor.  Row 0 is zeros so
    the padding index (-1 -> 0) contributes nothing to values or count.
  - SWDGE dma_gather pulls one 256B row per (pair, l) lookup: the Q7 pair
    generates 16 descriptors per vector push (~0.3 ns/desc) and the 16 SDMA
    engines execute them, so the gather runs at DMA rate instead of the
    ~33 cycles/index of the ap_gather compute path (which was 565 us of the
    673 us baseline).  5 phases (one per l) x 4 chunks of 8192 indices.
  - Gathered rows land [p = pos%128, pos//128, 0:64]; the host orders the
    index stream so stream position pos maps to pair q = (pos%128)*256 +
    pos//128, which makes each partition own 256 consecutive pairs - DVE
    accumulates the 8 head values + the valid flag (col 40) per phase with
    strided APs, and the final per-h output DMA is a clean [128, 1KB] store.
"""

import numpy as np

import concourse.bass as bass
import concourse.mybir as mybir
import concourse.tile as tile
from concourse import bacc
from concourse.bass_utils import run_bass_kernel_spmd

N, L, H, D, E = 512, 5, 8, 16, 10000
NCORES = 8
RPC = N // NCORES            # 64 rows (i) per core
NP = RPC * N                 # 32768 pairs per core
ROWS = E + 1                 # table rows (row 0 = padding zeros)
ELEM = 64                    # f32 per table row = 256B (dma_gather minimum)
WCOLS = 48                   # projection cols: 40 values + valid@40 + 7 pad
CH = 8192                    # indices per dma_gather call
NCH = NP // CH               # 4 chunks per phase
JB = CH // 128               # 64 col-blocks per chunk
CBS = NP // 128              # 256 col-blocks total per phase
f32, i16 = mybir.dt.float32, mybir.dt.int16

_cached = {}


def build_nc():
    nc = bacc.Bacc(dynamic_dma_scratch_size=32768)

    fw = nc.dram_tensor("fw", [D + 1, E + WCOLS], f32, kind="ExternalInput")
    idxw_t = nc.dram_tensor("idxw", [128, 5 * NP // 16], i16, kind="ExternalInput")
    tbl = nc.dram_tensor("tbl", [ROWS, ELEM], f32, kind="Internal")
    out = nc.dram_tensor("out", [H, RPC, N], f32, kind="ExternalOutput")

    with tile.TileContext(nc) as tc:
        with (
            tc.tile_pool(name="const", bufs=1) as cpool,
            tc.tile_pool(name="bld", bufs=3) as bpool,
            tc.tile_pool(name="mm", bufs=4, space="PSUM") as mmpool,
            tc.tile_pool(name="gth", bufs=3) as gpool,
            tc.tile_pool(name="ov", bufs=2) as opool,
        ):
            # tiny dummy gather against row 0 only: forces the mlp ucode
            # library load here, overlapped with the input DMAs below
            zi = cpool.tile([128, 1], i16)
            nc.vector.memset(zi[:, :], 0)
            zrow = cpool.tile([128, ELEM], f32)
            nc.vector.memset(zrow[:, :], 0.0)
            nc.sync.dma_start(out=tbl[0:1, :], in_=zrow[0:1, :])
            dgth = cpool.tile([128, ELEM], f32)
            nc.gpsimd.dma_gather(
                out_ap=dgth[:, :].rearrange("p (b e) -> p b e", e=ELEM),
                in_ap=tbl[0:1, :],
                idxs_ap=zi[:, :],
                num_idxs=16,
                num_idxs_reg=16,
                elem_size=ELEM,
                single_packet=False,
            )

            fw_sb = cpool.tile([D + 1, E + WCOLS], f32)
            nc.sync.dma_start(out=fw_sb[:, :], in_=fw[:, :])
            idxw = cpool.tile([128, 5 * NP // 16], i16)
            nc.sync.dma_start(out=idxw[:, :], in_=idxw_t[:, :])

            # ---- build TBL rows 1..E on PE: [e, (l,h)] = feats @ W ----
            for e0 in range(0, E, 128):
                m = min(128, E - e0)
                ps = mmpool.tile([128, WCOLS], f32, space="PSUM", tag="mm")
                nc.tensor.matmul(
                    out=ps[0:m, :],
                    lhsT=fw_sb[:, e0:e0 + m],
                    rhs=fw_sb[:, E:E + WCOLS],
                    start=True,
                    stop=True,
                )
                bt = bpool.tile([128, ELEM], f32, tag="bld")
                nc.vector.memset(bt[:, WCOLS:ELEM], 0.0)
                nc.vector.tensor_copy(out=bt[0:m, 0:WCOLS], in_=ps[0:m, :])
                nc.sync.dma_start(out=tbl[1 + e0:1 + e0 + m, :], in_=bt[0:m, :])

            acc = cpool.tile([128, CBS * H], f32)    # [p, cb*8+h]
            cnt = cpool.tile([128, CBS], f32)        # [p, cb]

            # ---- gather + accumulate ----
            for l in range(L):
                for c in range(NCH):
                    gth = gpool.tile([128, JB * ELEM], f32, tag="gth")
                    g3 = gth[:, :].rearrange("p (jb e) -> p jb e", e=ELEM)
                    i0 = l * (NP // 16) + c * (CH // 16)
                    nc.gpsimd.dma_gather(
                        out_ap=g3,
                        in_ap=tbl[:, :],
                        idxs_ap=idxw[:, i0:i0 + CH // 16],
                        num_idxs=CH,
                        num_idxs_reg=CH,
                        elem_size=ELEM,
                        # 513 concatenated descs/engine would blow the ≤64-desc
                        # packet ceiling; one packet per 256B descriptor
                        single_packet=False,
                    )
                    a3 = acc[:, c * JB * H:(c + 1) * JB * H].rearrange(
                        "p (jb e) -> p jb e", e=H
                    )
                    gv = g3[:, :, 8 * l:8 * l + H]
                    c2 = cnt[:, c * JB:(c + 1) * JB].rearrange(
                        "p (jb e) -> p jb e", e=1
                    )
                    gc = g3[:, :, 40:41]
                    if l == 0:
                        nc.vector.tensor_copy(out=a3, in_=gv)
                        nc.vector.tensor_copy(out=c2, in_=gc)
                    else:
                        nc.vector.tensor_add(out=a3, in0=a3, in1=gv)
                        nc.vector.tensor_add(out=c2, in0=c2, in1=gc)

            # ---- 1/max(count,1), scale, store ----
            nm = cpool.tile([128, CBS], f32)
            nc.vector.tensor_scalar_max(out=nm[:, :], in0=cnt[:, :], scalar1=1.0)
            rcp = cpool.tile([128, CBS], f32)
            rscr = cpool.tile([128, CBS], f32)
            nc.vector.reciprocal_approx_accurate(
                out=rcp[:, :], in_=nm[:, :], scratch=rscr[:, :]
            )
            a4 = acc[:, :].rearrange("p (cb e) -> p cb e", e=H)
            for h in range(H):
                ov = opool.tile([128, CBS], f32, tag="ov")
                nc.vector.tensor_tensor(
                    out=ov[:, :], in0=a4[:, :, h], in1=rcp[:, :],
                    op=mybir.AluOpType.mult,
                )
                nc.sync.dma_start(
                    out=out[h, :, :].rearrange("i (j2 c) -> (i j2) c", c=CBS),
                    in_=ov[:, :],
                )
    nc.compile()
    return nc


def _host_prep(edge_features_s, edge_weights, shortest_path_edges):
    feats = np.asarray(edge_features_s, dtype=np.float32)
    ew = np.asarray(edge_weights, dtype=np.float32)
    spe = np.asarray(shortest_path_edges).astype(np.int64)

    # fw = [featsT17 | WV]: featsT17 [17, E] = feats^T + ones row;
    # WV [17, 48]: col 8l+h = (W[l,h,:], 0); col 40 = (0.., 1); rest 0
    W = ew[1:L + 1].reshape(L, H, D)
    featsT17 = np.concatenate([feats.T, np.ones((1, E), np.float32)], axis=0)
    wv = np.zeros((D + 1, WCOLS), np.float32)
    for l in range(L):
        for h in range(H):
            wv[:D, 8 * l + h] = W[l, h]
    wv[D, 40] = 1.0
    fw = np.ascontiguousarray(np.concatenate([featsT17, wv], axis=1))

    comb = (spe + 1).astype(np.int32)   # [N, N, L], 0 = padding
    # per-device wrapped index streams: stream position pos covers pair
    # q = (pos%128)*256 + pos//128 so partition p owns pairs p*256..p*256+255
    idxw_all = np.zeros((NCORES, 128, 5 * NP // 16), np.int16)
    for cdev in range(NCORES):
        sub = comb[cdev * RPC:(cdev + 1) * RPC]            # [64, 512, L]
        for l in range(L):
            flat_q = sub[:, :, l].reshape(NP)              # q = i*512 + j
            stream = flat_q.reshape(128, CBS).T.reshape(NP)
            wrapped = stream.reshape(NP // 16, 16).T       # [16, NP/16]
            idxw_all[cdev, :, l * (NP // 16):(l + 1) * (NP // 16)] = np.tile(
                wrapped.astype(np.int16), (8, 1)
            )
    return fw, idxw_all


def kernel(edge_features_s, edge_weights, shortest_path_edges):
    if "nc" not in _cached:
        _cached["nc"] = build_nc()
    nc = _cached["nc"]

    fw, idxw_all = _host_prep(edge_features_s, edge_weights, shortest_path_edges)
    in_maps = []
    for c in range(NCORES):
        in_maps.append({
            "idxw": np.ascontiguousarray(idxw_all[c]),
            "fw": fw,
        })
    res = run_bass_kernel_spmd(nc, in_maps, list(range(NCORES)))
    outs = [res.results[c]["out"].reshape(H, RPC, N) for c in range(NCORES)]
    return np.concatenate(outs, axis=1)


# revision 6
# speedup vs baseline: 2.9875x; 2.9875x over previous
"""EdgeEncoder kernel for Trainium2 (8 NeuronCores, row-sharded).

Reference (per pair (i, j) of an N x N grid):
    out[h, i, j] = (1/n_ij) * sum_l mask[i,j,l] * sum_d feats[idx[i,j,l], d] * W[l, h, d]
with n_ij = max(#valid l, 1), idx in [-1, E-1], -1 = padding.

Device strategy (per core, which owns 64 rows i):
  - Projected tables T_l[e, h] = sum_d feats[e,d] W[l,h,d] are built on PE as
    [128 channels, e] tiles: channel (16g+c): c<8 -> value column h=c, c>=8 ->
    "validity" column (constant 1 via an appended ones-feature row).  Row 0 of
    each l-block is zeros (padding target).  Operands are bf16 (fp32 PSUM
    accumulate; end-to-end rel err ~2e-3 vs the 2e-2 gate) and the feature
    load is column-chunked so the phase-0 build starts ~5us in - the gather
    train launches at ~16us instead of ~70us.
  - gpsimd ap_gather: Q7 core g gathers the stream (pair in share_g) from the
    SBUF-resident table; all 16 channels of the core follow the stream, so
    values for all 8 heads AND the validity bit arrive in one pass.  One phase
    per l (5 tables of 10001 rows; int16 indices), table builds overlap the
    previous phase's gather.  ap_gather costs ~27.7ns/index/Q7-core and is the
    ~570us wall; the SWDGE dma_gather alternative measures ~9ns/index on only
    2 Q7 cores (73.6us per 8192 indices) and loses.
  - DVE reduces over l into acc[(g,c), (i_l, j)]; counts land on channels c>=8.
  - The last phase is gathered in 4 quarter chunks and the finale
    (recip(max(count,1)), partition-shift DMA, multiply, output DMAs) runs
    per quarter so only ~1/4 of it trails the final gather.
"""

import numpy as np
import ml_dtypes

import concourse.bass as bass
import concourse.mybir as mybir
import concourse.tile as tile
from concourse import bacc
from concourse.bass_utils import run_bass_kernel_spmd

N, L, H, D, E = 512, 5, 8, 16, 10000
NCORES = 8
RPC = N // NCORES            # 64 rows (i) per core
IPG = RPC // 8               # 8 rows (i) per Q7 core / share
PAIRS_G = IPG * N            # 4096 pairs per share
BLK = E + 1                  # 10001 rows per l-block (row 0 = zeros)
PCH = 2048                   # pairs per gather chunk (phases 0-3)
QCH = 1024                   # pairs per gather chunk (last phase + finale)
ECH = 512                    # e-chunk for table build
f32, i16 = mybir.dt.float32, mybir.dt.int16
bf16 = mybir.dt.bfloat16

IDXW_COLS = L * PAIRS_G // 16   # 1280 int16 cols per partition

_cached = {}


def build_nc():
    nc = bacc.Bacc()

    idxw_t = nc.dram_tensor("idxw", [128, IDXW_COLS], i16, kind="ExternalInput")
    fw = nc.dram_tensor("fw", [D + 1, E + 5 * 128], bf16, kind="ExternalInput")
    out = nc.dram_tensor("out", [H, RPC, N], f32, kind="ExternalOutput")

    with tile.TileContext(nc) as tc:
        with (
            tc.tile_pool(name="const", bufs=1) as cpool,
            tc.tile_pool(name="tbl", bufs=2) as tpool,
            tc.tile_pool(name="mm", bufs=2, space="PSUM") as mmpool,
            tc.tile_pool(name="gth", bufs=2) as gpool,
            tc.tile_pool(name="acc", bufs=1) as apool,
        ):
            # tiny dummy gather: forces the gpsimd ucode library load to
            # happen here, overlapped with the input DMAs below
            zi = cpool.tile([128, 1], i16)
            nc.vector.memset(zi[:, :], 0)
            zt = cpool.tile([128, 16], f32)
            nc.vector.memset(zt[:, :], 0.0)
            zo = cpool.tile([128, 16], f32)
            nc.gpsimd.ap_gather(
                out_ap=zo[:, :], in_ap=zt[:, :], idxs_ap=zi[:, :],
                channels=128, num_elems=16, d=1, num_idxs=16,
            )

            # weight columns first (every matmul needs them), then the
            # feature columns chunk-by-chunk so build-0 overlaps the load
            fw_sb = cpool.tile([D + 1, E + 5 * 128], bf16)
            nc.sync.dma_start(
                out=fw_sb[:, E:E + 5 * 128], in_=fw[:, E:E + 5 * 128]
            )
            for c0 in range(0, E, 4 * ECH):
                c1 = min(E, c0 + 4 * ECH)
                nc.sync.dma_start(out=fw_sb[:, c0:c1], in_=fw[:, c0:c1])
            idxw = cpool.tile([128, IDXW_COLS], i16)
            nc.sync.dma_start(out=idxw[:, :], in_=idxw_t[:, :])

            acc = apool.tile([128, PAIRS_G], f32)       # [(g,c), (il, j)]

            for l in range(L):
                tbl = tpool.tile([128, BLK], f32, tag="tbl")
                nc.vector.memset(tbl[:, 0:1], 0.0)
                for e0 in range(0, E, 4 * ECH):
                    bcnt = min(4 * ECH, E - e0)
                    ps = mmpool.tile([128, 4 * ECH], f32, space="PSUM", tag="mm")
                    for s0 in range(0, bcnt, ECH):
                        cnt = min(ECH, bcnt - s0)
                        nc.tensor.matmul(
                            out=ps[:, s0:s0 + cnt],
                            lhsT=fw_sb[:, E + l * 128:E + (l + 1) * 128],
                            rhs=fw_sb[:, e0 + s0:e0 + s0 + cnt],
                            start=True,
                            stop=True,
                        )
                    nc.vector.tensor_copy(
                        out=tbl[:, 1 + e0:1 + e0 + bcnt],
                        in_=ps[:, :bcnt],
                    )
                chunk = PCH if l < L - 1 else QCH
                for off in range(0, PAIRS_G, chunk):
                    gth = gpool.tile([128, PCH], f32, tag="gth")
                    c0 = l * (PAIRS_G // 16) + off // 16
                    nc.gpsimd.ap_gather(
                        out_ap=gth[:, 0:chunk],
                        in_ap=tbl[:, :],
                        idxs_ap=idxw[:, c0:c0 + chunk // 16],
                        channels=128,
                        num_elems=BLK,
                        d=1,
                        num_idxs=chunk,
                    )
                    asl = acc[:, off:off + chunk]
                    if l == 0:
                        nc.vector.tensor_copy(out=asl, in_=gth[:, 0:chunk])
                    else:
                        nc.vector.tensor_add(
                            out=asl, in0=asl, in1=gth[:, 0:chunk]
                        )

            # 1/max(count,1) lives on channels c>=8; shift to value channels.
            # Quarter-chunked and aligned with the last phase's gathers so
            # only the final quarter trails the last gather.
            rt = gpool.tile([128, PAIRS_G], f32, tag="gth")
            scratch = gpool.tile([128, PAIRS_G], f32, tag="gth")
            rt2 = apool.tile([128, PAIRS_G], f32)
            for ch in range(PAIRS_G // QCH):
                sl = slice(ch * QCH, (ch + 1) * QCH)
                nc.vector.tensor_scalar_max(
                    out=rt[:, sl], in0=acc[:, sl], scalar1=1.0
                )
                # ~2 ULP, ~2.8x faster than InstReciprocal (counts in [1,5])
                nc.vector.reciprocal_approx_accurate(
                    out=rt2[:, sl], in_=rt[:, sl], scratch=scratch[:, sl]
                )
                nc.sync.dma_start(out=scratch[0:120, sl], in_=rt2[8:128, sl])
                nc.vector.tensor_tensor(
                    out=rt[0:120, sl], in0=acc[0:120, sl],
                    in1=scratch[0:120, sl], op=mybir.AluOpType.mult,
                )
                ni = QCH // N  # i-rows per finale chunk (2)
                for g in range(8):
                    i0 = g * IPG + ch * ni
                    dst = out[:, i0:i0 + ni, :]
                    nc.sync.dma_start(
                        out=dst.rearrange("h i j -> h i j"),
                        in_=rt[16 * g:16 * g + H, sl].rearrange(
                            "c (i j) -> c i j", j=N
                        ),
                    )
    nc.compile()
    return nc


def _host_prep(edge_features_s, edge_weights, shortest_path_edges):
    feats = np.asarray(edge_features_s, dtype=np.float32)
    ew = np.asarray(edge_weights, dtype=np.float32)
    spe = np.asarray(shortest_path_edges).astype(np.int64)

    # fw = [featsT17 | W_CH]:
    #   featsT17 [17, E]: feats^T with an appended ones row
    #   W_CH [17, 5*128]: col l*128+p: p%16<8 -> (W[l, p%16, :], 0); else (0.., 1)
    W = ew[1:L + 1].reshape(L, H, D)
    featsT17 = np.concatenate([feats.T, np.ones((1, E), np.float32)], axis=0)
    wch = np.zeros((D + 1, 5 * 128), np.float32)
    for l in range(L):
        for p in range(128):
            c = p % 16
            if c < H:
                wch[:D, l * 128 + p] = W[l, c]
            else:
                wch[D, l * 128 + p] = 1.0
    fw = np.ascontiguousarray(
        np.concatenate([featsT17, wch], axis=1).astype(ml_dtypes.bfloat16)
    )

    comb = (spe + 1).astype(np.int32)   # [N, N, L], 0 = padding
    # per-device wrapped index streams
    idxw_all = np.zeros((NCORES, 128, IDXW_COLS), np.int16)
    for cdev in range(NCORES):
        sub = comb[cdev * RPC:(cdev + 1) * RPC]  # [64, 512, 5]
        for l in range(L):
            for g in range(8):
                flat = sub[g * IPG:(g + 1) * IPG][:, :, l].reshape(-1)
                wrapped = flat.reshape(PAIRS_G // 16, 16).T
                idxw_all[cdev, 16 * g:16 * g + 16,
                         l * (PAIRS_G // 16):(l + 1) * (PAIRS_G // 16)] = wrapped
    return fw, idxw_all


def kernel(edge_features_s, edge_weights, shortest_path_edges):
    if "nc" not in _cached:
        _cached["nc"] = build_nc()
    nc = _cached["nc"]

    fw, idxw_all = _host_prep(edge_features_s, edge_weights, shortest_path_edges)
    in_maps = []
    for c in range(NCORES):
        in_maps.append({
            "idxw": np.ascontiguousarray(idxw_all[c]),
            "fw": fw,
        })
    res = run_bass_kernel_spmd(nc, in_maps, list(range(NCORES)))
    outs = [res.results[c]["out"].reshape(H, RPC, N) for c in range(NCORES)]
    return np.concatenate(outs, axis=1)
